# revision 9
# baseline (speedup 1.0000x reference)
"""Multi-head attention (nn_Attention) for 8 Trainium2 NeuronCores.

Sharding: tensor-parallel over heads (2 heads per core). Each core computes
qkv projection for its head slice from the full input, full attention for its
2 heads, and a partial output projection; partials are summed on the host.

Layout strategy (per core):
  - qkv^T = W_slice @ x^T computed with contraction (c=1024) on the partition
    dim; produces q^T/k^T [128=2*64 head dims, tokens] directly in the
    orientation the S^T matmuls need.
  - S^T tiles [128 keys, 512 queries x 2 heads] via row-tiled matmul pairs
    (head A on array rows 0:63, head B on 64:127) which execute CONCURRENTLY
    on the PE's row groups.
  - softmax without max-subtraction (|S| < 7 for these inputs): exp on ACT
    (PSUM -> SBUF bf16), then O^T = (E^T [v|ones]) with the ones columns
    producing the softmax normalizer Z on the opposite 64 partitions.
  - Z rows are moved onto the O rows' partitions with a swap-halves
    permutation matmul, reciprocal via the fast custom DVE op, and the
    normalization is fused into the PSUM->SBUF copy (tensor_mul).
  - proj: out_partial[tokens, feat] = O^T_cat.T @ w_projT_slice, summed on
    host across cores. v-bias is folded into b_proj on the host (softmax
    weights sum to 1, so the v bias adds a constant to O).

Pipelining: the exp on ACT (1 elem/cycle/lane @1.2GHz) is the hard floor
(~1.12us per [128,1024] score tile). The inner loop is organized as uniform
per-key-tile slots matched to that cadence: each slot issues one S-pair, the
PV pair lagging two slots, and about one small filler item (a 2-matmul slice
of the next batch's qkv, or deferred projection tiles). The projection is
mostly deferred into the last batch's slots, which have no qkv filler work.
All matmul operands are bf16 (fp32 streams at ~2 cycles/row on HW, bf16 at
1); intermediates accumulate in fp32 PSUM. Output partials ship as bf16.
"""

import os
import numpy as np

N_CORES = 8
DIM = 1024
N_HEADS = 16
HEAD_DIM = 64
SCALE = HEAD_DIM ** -0.5
B, N = 4, 2048
TOK = B * N  # 8192
NB_C = DIM // 128   # 8 contraction tiles for qkv
NB_J = N // 128     # 16 key tiles per batch
NB_QC = N // 512    # 4 query chunks per batch
NB_TCH = N // 512   # 4 token chunks per batch (qkv)

_cache = {}


def _build():
    if "nc" in _cache:
        return _cache["nc"]
    import concourse.bacc as bacc
    import concourse.mybir as mybir
    from concourse.tile import TileContext

    f32 = mybir.dt.float32
    bf16 = mybir.dt.bfloat16
    Exp = mybir.ActivationFunctionType.Exp

    nc = bacc.Bacc(None, target_bir_lowering=False)
    xT_d = nc.dram_tensor("xT", [DIM, TOK], bf16, kind="ExternalInput")
    wqkvT_d = nc.dram_tensor("wqkvT", [DIM, 384], bf16, kind="ExternalInput")
    bias_d = nc.dram_tensor("bias", [128, 3], f32, kind="ExternalInput")
    wprojT_d = nc.dram_tensor("wprojT", [128, DIM], bf16, kind="ExternalInput")
    ident_d = nc.dram_tensor("ident", [128, 128], f32, kind="ExternalInput")
    swap_d = nc.dram_tensor("swap", [128, 128], bf16, kind="ExternalInput")
    out_d = nc.dram_tensor("out", [TOK, DIM], bf16, kind="ExternalOutput")

    with TileContext(nc) as tc:
        with tc.tile_pool(name="sbuf", bufs=1) as sb, \
             tc.tile_pool(name="psum", bufs=1, space="PSUM") as ps:
            # constants / weights
            wqkv_t = sb.tile([128, NB_C, 384], bf16, tag="wqkv")
            _wsrc = wqkvT_d[:, :].rearrange("(ct p) r -> p ct r", p=128)
            for ct in range(NB_C):
                nc.sync.dma_start(wqkv_t[:, ct:ct + 1, :], _wsrc[:, ct:ct + 1, :])
            wproj_t = sb.tile([128, DIM], bf16, tag="wproj")
            nc.sync.dma_start(wproj_t, wprojT_d[:, :])
            bias_t = sb.tile([128, 3], f32, tag="bias")
            nc.sync.dma_start(bias_t, bias_d[:, :])
            ident_f = sb.tile([128, 128], f32, tag="ident")
            nc.sync.dma_start(ident_f, ident_d[:, :])
            swap_t = sb.tile([128, 128], bf16, tag="swap")
            nc.sync.dma_start(swap_t, swap_d[:, :])
            ones_t = sb.tile([128, 1], bf16, tag="ones")
            nc.vector.memset(ones_t, 1.0)
            # preload the exp table set during the DMA lead-in
            warm_t = sb.tile([128, 1], f32, tag="warm")
            nc.scalar.activation(warm_t, ones_t, Exp)

            def alloc_batch_tiles():
                qT_t = sb.tile([128, N], bf16, tag="qT", bufs=2)
                kT_t = sb.tile([128, N], bf16, tag="kT", bufs=2)
                # v laid out [tok128, head, ktile, 128] with ones columns:
                # head A block cols = [v_A(64) | ones(64)], head B = [ones | v_B]
                v_t = sb.tile([128, 2, NB_J, 128], bf16, tag="v", bufs=2)
                nc.vector.tensor_copy(
                    v_t[:, 0, :, 64:128],
                    ones_t[:, None, :].broadcast_to([128, NB_J, 64]))
                nc.vector.tensor_copy(
                    v_t[:, 1, :, 0:64],
                    ones_t[:, None, :].broadcast_to([128, NB_J, 64]))
                return qT_t, kT_t, v_t

            def dma_xbatch(b, split=2):
                # stage all of batch b's x (one buffer per batch)
                xa = sb.tile([128, NB_TCH, NB_C, 512], bf16, tag="xa", bufs=2)
                for t in range(NB_TCH):
                    t0 = b * N + t * 512
                    src = (xT_d[:, t0:t0 + 512]
                           .rearrange("(ct p) t -> p ct t", p=128))
                    step = NB_C // split
                    for c0 in range(0, NB_C, step):
                        nc.sync.dma_start(xa[:, t, c0:c0 + step, :],
                                          src[:, c0:c0 + step, :])
                return xa

            def qkv_items(tiles_n, tch, r, xa):
                # one r-block (q/k/v projection for one 512-token chunk),
                # sliced into ~2-matmul filler items. r: 0=q, 1=k, 2=v.
                qT_n, kT_n, v_n = tiles_n
                state = {}
                items = []

                def mk_mm(c0):
                    def f():
                        if c0 == 0:
                            state["qp"] = ps.tile([128, 512], f32,
                                                  name="qp", tag="misc",
                                                  bufs=2)
                        qp = state["qp"]
                        for ct in (c0, c0 + 1):
                            nc.tensor.matmul(
                                qp, wqkv_t[:, ct, r * 128:(r + 1) * 128],
                                xa[:, tch, ct, :],
                                start=(ct == 0), stop=(ct == NB_C - 1))
                        if c0 == NB_C - 2:
                            if r == 0:
                                nc.vector.tensor_scalar_add(
                                    qT_n[:, tch * 512:(tch + 1) * 512], qp,
                                    bias_t[:, 0:1])
                            elif r == 1:
                                nc.vector.tensor_scalar_add(
                                    kT_n[:, tch * 512:(tch + 1) * 512], qp,
                                    bias_t[:, 1:2])
                            else:
                                vst = sb.tile([128, 512], f32, tag="vtst",
                                              bufs=2)
                                nc.vector.tensor_copy(vst, qp)
                                state["vst"] = vst
                    return f

                for c0 in range(0, NB_C, 2):
                    items.append(mk_mm(c0))
                if r == 2:
                    def mk_tr(s0):
                        def f():
                            vst = state["vst"]
                            for s in (s0, s0 + 1):
                                trp = ps.tile([128, 128], f32, tag="misc",
                                              bufs=2)
                                nc.tensor.transpose(
                                    trp, vst[:, s * 128:(s + 1) * 128],
                                    ident_f)
                                j = tch * 4 + s
                                nc.vector.tensor_copy(v_n[:, 0, j, 0:64],
                                                      trp[:, 0:64])
                                nc.vector.tensor_copy(v_n[:, 1, j, 64:128],
                                                      trp[:, 64:128])
                        return f
                    items += [mk_tr(0), mk_tr(2)]
                return items

            def proj_mm(ot_p, b_p, qc_p, idx):
                # one (ts, fc) output tile of the deferred projection
                ts, fc = divmod(idx, 2)
                pj = ps.tile([128, 512], f32, tag="misc", bufs=2)
                nc.tensor.matmul(
                    pj, ot_p[:, ts * 128:(ts + 1) * 128],
                    wproj_t[:, fc * 512:(fc + 1) * 512], start=True, stop=True)
                ost = sb.tile([128, 512], bf16, tag="ost", bufs=4)
                nc.vector.tensor_copy(ost, pj)
                row0 = b_p * N + qc_p * 512 + ts * 128
                nc.sync.dma_start(
                    out_d[row0:row0 + 128, fc * 512:(fc + 1) * 512], ost)

            def finish_norm(pending):
                # swap Z rows onto O rows' lanes via a permutation matmul,
                # then reciprocal + normalization fused into the PSUM drain.
                z_p, oA_p, oB_p, b_p, qc_p = pending
                zsw = ps.tile([128, 512], f32, tag="misc", bufs=2)
                nc.tensor.matmul(zsw, swap_t, z_p, start=True, stop=True)
                r_t = sb.tile([128, 512], f32, tag="rt", bufs=2)
                nc.vector.reciprocal_approx_fast(out=r_t, in_=zsw)
                ot = sb.tile([128, 512], bf16, tag="ot", bufs=8)
                nc.vector.tensor_mul(ot[0:64, :], oA_p[0:64, :], r_t[0:64, :])
                nc.vector.tensor_mul(ot[64:128, :], oB_p[64:128, :],
                                     r_t[64:128, :])
                return (ot, b_p, qc_p)

            # ---- prologue: stage batch 0's x, compute its full QKV ----
            xa_cur = dma_xbatch(0, split=8)
            tiles = alloc_batch_tiles()
            for r in (1, 2, 0):          # k first (S needs all of k), then v, q
                for tch in range(NB_TCH):
                    for it in qkv_items(tiles, tch, r, xa_cur):
                        it()

            proj_queue = []  # deferred projection tile closures
            pending = None   # deferred normalization: (z, oA, oB, b, qc)
            xa_next = dma_xbatch(1) if B > 1 else None
            for b in range(B):
                qT_t, kT_t, v_t = tiles
                if b + 1 < B:
                    next_tiles = alloc_batch_tiles()
                for qc in range(NB_QC):
                    if qc == 1 and b + 2 < B:
                        xa_follow = dma_xbatch(b + 2)
                    # next batch's qkv for this token chunk, as filler items
                    fillers = []
                    if b + 1 < B:
                        for r in (1, 2, 0):
                            fillers += qkv_items(next_tiles, qc, r, xa_next)
                    q_sl = slice(qc * 512, (qc + 1) * 512)
                    oA = ps.tile([128, 512], f32, tag="oA", bufs=1)
                    oB = ps.tile([128, 512], f32, tag="oB", bufs=1)
                    e_pend = [None] * NB_J
                    for s in range(NB_J + 2):
                        if s < NB_J:
                            k_sl = slice(s * 128, (s + 1) * 128)
                            st = ps.tile([128, 1024], f32, tag="st", bufs=2)
                            nc.tensor.matmul(
                                st[:, 0:512], kT_t[0:64, k_sl],
                                qT_t[0:64, q_sl], start=True, stop=True)
                            nc.tensor.matmul(
                                st[:, 512:1024], kT_t[64:128, k_sl],
                                qT_t[64:128, q_sl], start=True, stop=True,
                                tile_position=(64, 0))
                            e_t = sb.tile([128, 1024], bf16, tag="e", bufs=6)
                            nc.scalar.activation(e_t, st, Exp)
                            e_pend[s] = e_t
                        if s == 0 and pending is not None:
                            ot_p, b_p, qc_p = finish_norm(pending)
                            pending = None
                            proj_queue += [
                                (lambda o=ot_p, bb=b_p, qq=qc_p, ii=i:
                                 proj_mm(o, bb, qq, ii)) for i in range(8)]
                        if s >= 2:
                            j = s - 2
                            e_p = e_pend[j]
                            nc.tensor.matmul(
                                oA, v_t[:, 0, j, :], e_p[:, 0:512],
                                start=(j == 0), stop=(j == NB_J - 1))
                            nc.tensor.matmul(
                                oB, v_t[:, 1, j, :], e_p[:, 512:1024],
                                start=(j == 0), stop=(j == NB_J - 1))
                        if fillers:
                            fillers.pop(0)()
                        else:
                            allow = 2 if (b == B - 1 or s >= NB_J) else 0
                            while allow and proj_queue:
                                proj_queue.pop(0)()
                                allow -= 1
                    # stage Z into SBUF now (DVE); the rest of the
                    # normalization happens early next qc.
                    z_st = sb.tile([128, 512], bf16, tag="zst", bufs=2)
                    nc.vector.tensor_copy(z_st[64:128, :], oA[64:128, :])
                    nc.vector.tensor_copy(z_st[0:64, :], oB[0:64, :])
                    pending = (z_st, oA, oB, b, qc)
                if b + 1 < B:
                    tiles = next_tiles
                    xa_cur = xa_next
                    xa_next = xa_follow if b + 2 < B else None
            # tail: last qc's normalization + remaining projection tiles
            ot_p, b_p, qc_p = finish_norm(pending)
            proj_queue += [
                (lambda o=ot_p, bb=b_p, qq=qc_p, ii=i: proj_mm(o, bb, qq, ii))
                for i in range(8)]
            for f in proj_queue:
                f()

    nc.compile()
    _cache["nc"] = nc
    return nc


def _ensure_ntff_hook():
    """Register the axon NTFF profile hook (antenv.axon_hooks) if absent.

    The agent image's antenv stub lacks axon_hooks, so trn_boot's hook
    registration silently degrades; recreate it here via the same ctypes
    recipe so run_bass_kernel_spmd(trace=True) can capture HW profiles.
    """
    import sys
    import types
    import ctypes
    import contextlib

    try:
        from antenv.axon_hooks import get_axon_ntff_profile_hook
        if get_axon_ntff_profile_hook() is not None:
            return
    except ImportError:
        mod = types.ModuleType("antenv.axon_hooks")
        mod._hook = None
        mod.get_axon_ntff_profile_hook = lambda: mod._hook

        def _set(h):
            mod._hook = h
        mod.set_axon_ntff_profile_hook = _set
        sys.modules["antenv.axon_hooks"] = mod
        import antenv
        antenv.axon_hooks = mod

    so_path = "/opt/axon/libaxon_pjrt.so"
    if not os.path.exists(so_path):
        return
    lib = ctypes.CDLL(so_path)
    if not hasattr(lib, "axon_start_nrt_profile"):
        return
    lib.axon_start_nrt_profile.argtypes = [
        ctypes.POINTER(ctypes.c_int64), ctypes.c_size_t]
    lib.axon_start_nrt_profile.restype = ctypes.c_int64
    lib.axon_stop_nrt_profile.argtypes = [ctypes.c_char_p]
    lib.axon_stop_nrt_profile.restype = ctypes.c_int64

    @contextlib.contextmanager
    def _hook(output_dir, device_ids):
        # the .so's GLOBAL_CLIENT is only set once something executes
        import jax
        jax.block_until_ready(
            jax.jit(lambda a: a + 1)(jax.numpy.zeros((8,), jax.numpy.float32)))
        if device_ids:
            ids = (ctypes.c_int64 * len(device_ids))(*device_ids)
            rc = lib.axon_start_nrt_profile(ids, len(device_ids))
        else:
            rc = lib.axon_start_nrt_profile(None, 0)
        if rc != 0:
            raise RuntimeError(f"axon_start_nrt_profile rc={rc}")
        try:
            yield
        finally:
            n = lib.axon_stop_nrt_profile(str(output_dir).encode())
            print(f"profile: {n} file(s) written to {output_dir}")

    from antenv.axon_hooks import set_axon_ntff_profile_hook
    set_axon_ntff_profile_hook(_hook)


def kernel(x, w_qkv, b_qkv, w_proj, b_proj):
    import ml_dtypes
    from concourse.bass_utils import run_bass_kernel_spmd

    bf16 = ml_dtypes.bfloat16
    nc = _build()
    x = np.asarray(x, dtype=np.float32)
    w_qkv = np.asarray(w_qkv, dtype=np.float32)
    b_qkv = np.asarray(b_qkv, dtype=np.float32)
    w_proj = np.asarray(w_proj, dtype=np.float32)
    b_proj = np.asarray(b_proj, dtype=np.float32)

    xT = np.ascontiguousarray(x.reshape(TOK, DIM).T).astype(bf16)
    ident = np.eye(128, dtype=np.float32)
    # permutation that swaps partition halves: out = swap.T @ z
    swap = np.zeros((128, 128), dtype=np.float32)
    swap[np.arange(64) + 64, np.arange(64)] = 1.0
    swap[np.arange(64), np.arange(64) + 64] = 1.0
    swap = swap.astype(bf16)

    in_maps = []
    for c in range(N_CORES):
        sl = slice(HEAD_DIM * 2 * c, HEAD_DIM * 2 * c + 128)
        wq = w_qkv[0 * DIM:1 * DIM][sl] * SCALE
        wk = w_qkv[1 * DIM:2 * DIM][sl]
        wv = w_qkv[2 * DIM:3 * DIM][sl]
        wqkvT = np.ascontiguousarray(
            np.concatenate([wq, wk, wv], 0).T).astype(bf16)
        bq = b_qkv[0 * DIM:1 * DIM][sl] * SCALE
        bk = b_qkv[1 * DIM:2 * DIM][sl]
        bias = np.ascontiguousarray(
            np.stack([bq, bk, np.zeros_like(bq)], 1))
        wprojT = np.ascontiguousarray(w_proj[:, sl].T).astype(bf16)
        in_maps.append({"xT": xT, "wqkvT": wqkvT, "bias": bias,
                        "wprojT": wprojT, "ident": ident, "swap": swap})

    trace = os.environ.get("BASS_KERNEL_TRACE", "0") == "1"
    if trace:
        _ensure_ntff_hook()
    res = run_bass_kernel_spmd(nc, in_maps, list(range(N_CORES)), trace=trace)
    if trace:
        _cache["last_exec_time_ns"] = res.exec_time_ns
        _cache["last_mean_exec_time_ns"] = res.mean_exec_time_ns

    out = np.zeros((TOK, DIM), dtype=np.float64)
    for c in range(N_CORES):
        out += res.results[c]["out"].astype(np.float64)
    # v-bias contributes a constant (softmax weights sum to 1): fold into
    # the projection bias here instead of adding it on-device.
    out += b_proj + b_qkv[2 * DIM:3 * DIM] @ w_proj.T
    return out.reshape(B, N, DIM).astype(np.float32)


# revision 10
# speedup vs baseline: 1.1599x; 1.1599x over previous
"""Multi-head attention (nn_Attention) for 8 Trainium2 NeuronCores.

Sharding: tensor-parallel over heads (2 heads per core). Each core computes
qkv projection for its head slice from the full input, full attention for its
2 heads, and a partial output projection; partials are summed on the host.

Layout strategy (per core):
  - qkv^T = W_slice @ x^T computed with contraction (c=1024) on the partition
    dim; produces q^T/k^T [128=2*64 head dims, tokens] directly in the
    orientation the S^T matmuls need.
  - S^T tiles [128 keys, 512 queries x 2 heads] via row-tiled matmul pairs
    (head A on array rows 0:63, head B on 64:127) which execute CONCURRENTLY
    on the PE's row groups.
  - softmax without max-subtraction (|S| < 7 for these inputs): exp on ACT
    (PSUM -> SBUF bf16), then O^T = (E^T [v|ones]) with the ones columns
    producing the softmax normalizer Z on the opposite 64 partitions.
  - Z rows are moved onto the O rows' partitions with a swap-halves
    permutation matmul, reciprocal via the fast custom DVE op, and the
    normalization fused into the PSUM->SBUF copy (tensor_mul). The chain is
    deferred into the next qc's first slot so it never blocks the S/exp
    pipeline at qc boundaries.
  - proj: out_partial[tokens, feat] = O^T_cat.T @ w_projT_slice, summed on
    host across cores. v-bias is folded into b_proj on the host (softmax
    weights sum to 1, so the v bias adds a constant to O).
All matmul operands are bf16 (fp32 streams at ~2 cycles/row on HW, bf16 at
1); intermediates accumulate in fp32 PSUM. Output partials ship as bf16.
"""

import os
import numpy as np

N_CORES = 8
DIM = 1024
N_HEADS = 16
HEAD_DIM = 64
SCALE = HEAD_DIM ** -0.5
B, N = 4, 2048
TOK = B * N  # 8192
NB_C = DIM // 128   # 8 contraction tiles for qkv
NB_J = N // 128     # 16 key tiles per batch
NB_QC = N // 512    # 4 query chunks per batch
NB_TCH = N // 512   # 4 token chunks per batch (qkv)

_cache = {}


def _build():
    if "nc" in _cache:
        return _cache["nc"]
    import concourse.bacc as bacc
    import concourse.mybir as mybir
    from concourse.tile import TileContext

    f32 = mybir.dt.float32
    bf16 = mybir.dt.bfloat16
    Exp = mybir.ActivationFunctionType.Exp

    nc = bacc.Bacc(None, target_bir_lowering=False)
    xT_d = nc.dram_tensor("xT", [DIM, TOK], bf16, kind="ExternalInput")
    wqkvT_d = nc.dram_tensor("wqkvT", [DIM, 384], bf16, kind="ExternalInput")
    bias_d = nc.dram_tensor("bias", [128, 3], f32, kind="ExternalInput")
    wprojT_d = nc.dram_tensor("wprojT", [128, DIM], bf16, kind="ExternalInput")
    ident_d = nc.dram_tensor("ident", [128, 128], f32, kind="ExternalInput")
    swap_d = nc.dram_tensor("swap", [128, 128], bf16, kind="ExternalInput")
    out_d = nc.dram_tensor("out", [TOK, DIM], bf16, kind="ExternalOutput")

    with TileContext(nc) as tc:
        with tc.tile_pool(name="sbuf", bufs=1) as sb, \
             tc.tile_pool(name="psum", bufs=1, space="PSUM") as ps:
            # constants / weights
            wqkv_t = sb.tile([128, NB_C, 384], bf16, tag="wqkv")
            _wsrc = wqkvT_d[:, :].rearrange("(ct p) r -> p ct r", p=128)
            for ct in range(NB_C):
                nc.sync.dma_start(wqkv_t[:, ct:ct + 1, :], _wsrc[:, ct:ct + 1, :])
            wproj_t = sb.tile([128, DIM], bf16, tag="wproj")
            nc.sync.dma_start(wproj_t, wprojT_d[:, :])
            bias_t = sb.tile([128, 3], f32, tag="bias")
            nc.sync.dma_start(bias_t, bias_d[:, :])
            ident_f = sb.tile([128, 128], f32, tag="ident")
            nc.sync.dma_start(ident_f, ident_d[:, :])
            swap_t = sb.tile([128, 128], bf16, tag="swap")
            nc.sync.dma_start(swap_t, swap_d[:, :])
            ones_t = sb.tile([128, 1], bf16, tag="ones")
            nc.vector.memset(ones_t, 1.0)
            # preload the exp table set during the DMA lead-in
            warm_t = sb.tile([128, 1], f32, tag="warm")
            nc.scalar.activation(warm_t, ones_t, Exp)

            def alloc_batch_tiles():
                qT_t = sb.tile([128, N], bf16, tag="qT", bufs=2)
                kT_t = sb.tile([128, N], bf16, tag="kT", bufs=2)
                # v laid out [tok128, head, ktile, 128] with ones columns:
                # head A block cols = [v_A(64) | ones(64)], head B = [ones | v_B]
                v_t = sb.tile([128, 2, NB_J, 128], bf16, tag="v", bufs=2)
                nc.vector.tensor_copy(
                    v_t[:, 0, :, 64:128],
                    ones_t[:, None, :].broadcast_to([128, NB_J, 64]))
                nc.vector.tensor_copy(
                    v_t[:, 1, :, 0:64],
                    ones_t[:, None, :].broadcast_to([128, NB_J, 64]))
                return qT_t, kT_t, v_t

            def dma_xstage(b, tch, split=1):
                xst = sb.tile([128, NB_C, 512], bf16, tag="xst", bufs=4)
                t0 = b * N + tch * 512
                src = (xT_d[:, t0:t0 + 512]
                       .rearrange("(ct p) t -> p ct t", p=128))
                step = NB_C // split
                for c0 in range(0, NB_C, step):
                    nc.sync.dma_start(xst[:, c0:c0 + step, :],
                                      src[:, c0:c0 + step, :])
                return xst

            def qkv_r_block(tiles, tch, r, xst):
                qT_t, kT_t, v_t = tiles
                qp = ps.tile([128, 512], f32, tag="misc", bufs=2)
                for ct in range(NB_C):
                    nc.tensor.matmul(
                        qp, wqkv_t[:, ct, r * 128:(r + 1) * 128],
                        xst[:, ct, :],
                        start=(ct == 0), stop=(ct == NB_C - 1))
                if r == 0:
                    nc.vector.tensor_scalar_add(
                        qT_t[:, tch * 512:(tch + 1) * 512], qp, bias_t[:, 0:1])
                elif r == 1:
                    nc.vector.tensor_scalar_add(
                        kT_t[:, tch * 512:(tch + 1) * 512], qp, bias_t[:, 1:2])
                else:
                    vt_st = sb.tile([128, 512], f32, tag="vtst", bufs=2)
                    nc.vector.tensor_copy(vt_st, qp)
                    for s in range(4):
                        trp = ps.tile([128, 128], f32, tag="misc", bufs=2)
                        nc.tensor.transpose(
                            trp, vt_st[:, s * 128:(s + 1) * 128], ident_f)
                        j = tch * 4 + s
                        nc.vector.tensor_copy(v_t[:, 0, j, 0:64], trp[:, 0:64])
                        nc.vector.tensor_copy(v_t[:, 1, j, 64:128],
                                              trp[:, 64:128])

            def proj_mm(prev, idx):
                # one (ts, fc) output tile of the deferred projection
                ot_p, b_p, qc_p = prev
                ts, fc = divmod(idx, 2)
                pj = ps.tile([128, 512], f32, tag="misc", bufs=2)
                nc.tensor.matmul(
                    pj, ot_p[:, ts * 128:(ts + 1) * 128],
                    wproj_t[:, fc * 512:(fc + 1) * 512], start=True, stop=True)
                ost = sb.tile([128, 512], bf16, tag="ost", bufs=4)
                nc.vector.tensor_copy(ost, pj)
                row0 = b_p * N + qc_p * 512 + ts * 128
                nc.sync.dma_start(
                    out_d[row0:row0 + 128, fc * 512:(fc + 1) * 512], ost)

            def finish_norm(pending):
                # swap Z rows onto O rows' lanes via a permutation matmul,
                # then reciprocal + normalization fused into the PSUM drain.
                z_p, oA_p, oB_p, b_p, qc_p = pending
                zsw = ps.tile([128, 512], f32, tag="misc", bufs=2)
                nc.tensor.matmul(zsw, swap_t, z_p, start=True, stop=True)
                r_t = sb.tile([128, 512], f32, tag="rt", bufs=2)
                nc.vector.reciprocal_approx_fast(out=r_t, in_=zsw)
                ot = sb.tile([128, 512], bf16, tag="ot", bufs=2)
                nc.vector.tensor_mul(ot[0:64, :], oA_p[0:64, :], r_t[0:64, :])
                nc.vector.tensor_mul(ot[64:128, :], oB_p[64:128, :],
                                     r_t[64:128, :])
                return (ot, b_p, qc_p)

            # ---- prologue: full QKV for batch 0 ----
            tiles = alloc_batch_tiles()
            xsts = [dma_xstage(0, t, split=(8 if t == 0 else 2))
                    for t in range(NB_TCH)]
            for tch in range(NB_TCH):
                for r in range(3):
                    qkv_r_block(tiles, tch, r, xsts[tch])

            prev = None      # deferred projection: (ot, b, qc)
            pending = None   # deferred normalization: (z, oA, oB, b, qc)
            xst_q = dma_xstage(1, 0) if B > 1 else None
            for b in range(B):
                qT_t, kT_t, v_t = tiles
                if b + 1 < B:
                    next_tiles = alloc_batch_tiles()
                for qc in range(NB_QC):
                    # x chunk for this qc's qkv filler was prefetched one qc
                    # ago; issue the following one now.
                    xst_cur = xst_q
                    nb, nqc = (b, qc + 1) if qc + 1 < NB_QC else (b + 1, 0)
                    xst_q = (dma_xstage(nb + 1, nqc)
                             if nb + 1 < B else None)
                    q_sl = slice(qc * 512, (qc + 1) * 512)
                    oA = ps.tile([128, 512], f32, tag="oA", bufs=1)
                    oB = ps.tile([128, 512], f32, tag="oB", bufs=1)
                    # j-loop at key-tile-PAIR granularity: grouping the
                    # 64-row QK MMs and 128-row PV MMs reduces row-shape
                    # transitions so LDWEIGHTS hides under matmuls. Previous
                    # qc's proj and next batch's qkv chunks fill PE while ACT
                    # runs exp.
                    NPAIR = NB_J // 2
                    e_pend = [None] * NB_J
                    for m in range(NPAIR + 1):
                        if m < NPAIR:
                            for j in (2 * m, 2 * m + 1):
                                k_sl = slice(j * 128, (j + 1) * 128)
                                st = ps.tile([128, 1024], f32, tag="st",
                                             bufs=2)
                                nc.tensor.matmul(
                                    st[:, 0:512], kT_t[0:64, k_sl],
                                    qT_t[0:64, q_sl], start=True, stop=True)
                                nc.tensor.matmul(
                                    st[:, 512:1024], kT_t[64:128, k_sl],
                                    qT_t[64:128, q_sl],
                                    start=True, stop=True,
                                    tile_position=(64, 0))
                                e_t = sb.tile([128, 1024], bf16, tag="e",
                                              bufs=6)
                                nc.scalar.activation(e_t, st, Exp)
                                e_pend[j] = e_t
                        if m == 0 and pending is not None:
                            prev = finish_norm(pending)
                            pending = None
                        if prev is not None and 1 <= m <= 8:
                            proj_mm(prev, m - 1)
                        if b + 1 < B and m in (0, 3, 7):
                            qkv_r_block(next_tiles, qc, {0: 0, 3: 1, 7: 2}[m],
                                        xst_cur)
                        if m >= 1:
                            for j in (2 * m - 2, 2 * m - 1):
                                e_p = e_pend[j]
                                nc.tensor.matmul(
                                    oA, v_t[:, 0, j, :], e_p[:, 0:512],
                                    start=(j == 0), stop=(j == NB_J - 1))
                                nc.tensor.matmul(
                                    oB, v_t[:, 1, j, :], e_p[:, 512:1024],
                                    start=(j == 0), stop=(j == NB_J - 1))
                    # stage Z into SBUF now (DVE); the rest of the
                    # normalization happens early next qc.
                    z_st = sb.tile([128, 512], bf16, tag="zst", bufs=2)
                    nc.vector.tensor_copy(z_st[64:128, :], oA[64:128, :])
                    nc.vector.tensor_copy(z_st[0:64, :], oB[0:64, :])
                    pending = (z_st, oA, oB, b, qc)
                if b + 1 < B:
                    tiles = next_tiles
            # tail: last qc's normalization + projection
            prev = finish_norm(pending)
            for idx in range(8):
                proj_mm(prev, idx)

    nc.compile()
    _cache["nc"] = nc
    return nc


def _ensure_ntff_hook():
    """Register the axon NTFF profile hook (antenv.axon_hooks) if absent.

    The agent image's antenv stub lacks axon_hooks, so trn_boot's hook
    registration silently degrades; recreate it here via the same ctypes
    recipe so run_bass_kernel_spmd(trace=True) can capture HW profiles.
    """
    import sys
    import types
    import ctypes
    import contextlib

    try:
        from antenv.axon_hooks import get_axon_ntff_profile_hook
        if get_axon_ntff_profile_hook() is not None:
            return
    except ImportError:
        mod = types.ModuleType("antenv.axon_hooks")
        mod._hook = None
        mod.get_axon_ntff_profile_hook = lambda: mod._hook

        def _set(h):
            mod._hook = h
        mod.set_axon_ntff_profile_hook = _set
        sys.modules["antenv.axon_hooks"] = mod
        import antenv
        antenv.axon_hooks = mod

    so_path = "/opt/axon/libaxon_pjrt.so"
    if not os.path.exists(so_path):
        return
    lib = ctypes.CDLL(so_path)
    if not hasattr(lib, "axon_start_nrt_profile"):
        return
    lib.axon_start_nrt_profile.argtypes = [
        ctypes.POINTER(ctypes.c_int64), ctypes.c_size_t]
    lib.axon_start_nrt_profile.restype = ctypes.c_int64
    lib.axon_stop_nrt_profile.argtypes = [ctypes.c_char_p]
    lib.axon_stop_nrt_profile.restype = ctypes.c_int64

    @contextlib.contextmanager
    def _hook(output_dir, device_ids):
        # the .so's GLOBAL_CLIENT is only set once something executes
        import jax
        jax.block_until_ready(
            jax.jit(lambda a: a + 1)(jax.numpy.zeros((8,), jax.numpy.float32)))
        if device_ids:
            ids = (ctypes.c_int64 * len(device_ids))(*device_ids)
            rc = lib.axon_start_nrt_profile(ids, len(device_ids))
        else:
            rc = lib.axon_start_nrt_profile(None, 0)
        if rc != 0:
            raise RuntimeError(f"axon_start_nrt_profile rc={rc}")
        try:
            yield
        finally:
            n = lib.axon_stop_nrt_profile(str(output_dir).encode())
            print(f"profile: {n} file(s) written to {output_dir}")

    from antenv.axon_hooks import set_axon_ntff_profile_hook
    set_axon_ntff_profile_hook(_hook)


def kernel(x, w_qkv, b_qkv, w_proj, b_proj):
    import ml_dtypes
    from concourse.bass_utils import run_bass_kernel_spmd

    bf16 = ml_dtypes.bfloat16
    nc = _build()
    x = np.asarray(x, dtype=np.float32)
    w_qkv = np.asarray(w_qkv, dtype=np.float32)
    b_qkv = np.asarray(b_qkv, dtype=np.float32)
    w_proj = np.asarray(w_proj, dtype=np.float32)
    b_proj = np.asarray(b_proj, dtype=np.float32)

    xT = np.ascontiguousarray(x.reshape(TOK, DIM).T).astype(bf16)
    ident = np.eye(128, dtype=np.float32)
    # permutation that swaps partition halves: out = swap.T @ z
    swap = np.zeros((128, 128), dtype=np.float32)
    swap[np.arange(64) + 64, np.arange(64)] = 1.0
    swap[np.arange(64), np.arange(64) + 64] = 1.0
    swap = swap.astype(bf16)

    in_maps = []
    for c in range(N_CORES):
        sl = slice(HEAD_DIM * 2 * c, HEAD_DIM * 2 * c + 128)
        wq = w_qkv[0 * DIM:1 * DIM][sl] * SCALE
        wk = w_qkv[1 * DIM:2 * DIM][sl]
        wv = w_qkv[2 * DIM:3 * DIM][sl]
        wqkvT = np.ascontiguousarray(
            np.concatenate([wq, wk, wv], 0).T).astype(bf16)
        bq = b_qkv[0 * DIM:1 * DIM][sl] * SCALE
        bk = b_qkv[1 * DIM:2 * DIM][sl]
        bias = np.ascontiguousarray(
            np.stack([bq, bk, np.zeros_like(bq)], 1))
        wprojT = np.ascontiguousarray(w_proj[:, sl].T).astype(bf16)
        in_maps.append({"xT": xT, "wqkvT": wqkvT, "bias": bias,
                        "wprojT": wprojT, "ident": ident, "swap": swap})

    trace = os.environ.get("BASS_KERNEL_TRACE", "0") == "1"
    if trace:
        _ensure_ntff_hook()
    res = run_bass_kernel_spmd(nc, in_maps, list(range(N_CORES)), trace=trace)
    if trace:
        _cache["last_exec_time_ns"] = res.exec_time_ns
        _cache["last_mean_exec_time_ns"] = res.mean_exec_time_ns

    out = np.zeros((TOK, DIM), dtype=np.float64)
    for c in range(N_CORES):
        out += res.results[c]["out"].astype(np.float64)
    # v-bias contributes a constant (softmax weights sum to 1): fold into
    # the projection bias here instead of adding it on-device.
    out += b_proj + b_qkv[2 * DIM:3 * DIM] @ w_proj.T
    return out.reshape(B, N, DIM).astype(np.float32)
